# revision 2
# baseline (speedup 1.0000x reference)
"""Trainium2 Bass kernel for nn_DiffusionModel (theta_post_prob).

Math (per batch b, with runtime scalars a = alphas-gather, ca = cumalphas-gather):
    p     = a*xt + k1                 k1 = (1-a)/C
    M     = ca*I + u*ones             u  = (1-ca)/C   (C x C, symmetric, stochastic)
    denom = M^T p = a*(M^T xt) + k1   (column sums of M are 1)
    g     = theta_x0 / denom
    out   = p * (M g)

Kernel layout: batch b -> core b (pure data parallel, 8 cores). Per core the
(C=32, HW=65536) slab is processed as [128, N] tiles where the 128 partitions
pack G=4 independent spatial blocks x 32 classes. Both class-reductions
(+ their broadcasts + the diagonal term) are single PE matmuls against
block-diagonal 128x128 matrices kron(I4, a*M) / kron(I4, M) built on host.
"""

import sys

if "/opt/trn_rl_repo" not in sys.path:
    sys.path.insert(0, "/opt/trn_rl_repo")

import numpy as np

import concourse.bacc as bacc
import concourse.mybir as mybir
from concourse.tile import TileContext
from concourse.bass_utils import run_bass_kernel_spmd

F32 = mybir.dt.float32

T = 1000
C = 32
B = 8
H = 256
W = 256
HW = H * W

NCORES = 8
G = 4                 # spatial blocks packed into the 128 partitions
P = G * C             # 128
COLS = HW // G        # 16384 columns per spatial block
NT = 1024             # tile free-dim (per-DMA: 128 * NT * 4 B = 512 KiB)
ITERS = COLS // NT
MM_N = 512            # max moving free-dim for fp32 matmul

# Division strategy: "recip" = ACT bias-add + DVE reciprocal_approx_fast
#                    "lnexp" = ACT Ln(+k1) then ACT Exp(-x), no DVE recip
DIV_MODE = "recip"
# Matmul input dtype: "f32" (exact, 4 cyc/row) or "f32r" (1 cyc/row, ~tf32)
MM_DTYPE = "f32"

_CACHE = {}


def _build():
    key = (DIV_MODE, MM_DTYPE, NT)
    if key in _CACHE:
        return _CACHE[key]

    nc = bacc.Bacc(
        "TRN2",
        target_bir_lowering=False,
        debug=False,
        enable_asserts=False,
        num_devices=NCORES,
    )

    xt_d = nc.dram_tensor("xt", [C, HW], F32, kind="ExternalInput")
    x0_d = nc.dram_tensor("x0", [C, HW], F32, kind="ExternalInput")
    ma_d = nc.dram_tensor("ma", [P, P], F32, kind="ExternalInput")
    mb_d = nc.dram_tensor("mb", [P, P], F32, kind="ExternalInput")
    sc_d = nc.dram_tensor("sc", [P, 2], F32, kind="ExternalInput")
    out_d = nc.dram_tensor("out", [C, HW], F32, kind="ExternalOutput")

    xt_v = xt_d[:, :].rearrange("c (g n) -> g c n", g=G)
    x0_v = x0_d[:, :].rearrange("c (g n) -> g c n", g=G)
    out_v = out_d[:, :].rearrange("c (g n) -> g c n", g=G)

    AF = mybir.ActivationFunctionType

    with TileContext(nc) as tc:
        with (
            tc.tile_pool(name="consts", bufs=1) as cpool,
            tc.tile_pool(name="work", bufs=3) as pool,
            tc.tile_pool(name="psum", bufs=2, space="PSUM") as psum,
        ):
            ma = cpool.tile([P, P], F32)
            nc.sync.dma_start(ma[:, :], ma_d[:, :])
            mb = cpool.tile([P, P], F32)
            nc.sync.dma_start(mb[:, :], mb_d[:, :])
            sc = cpool.tile([P, 2], F32)
            nc.sync.dma_start(sc[:, :], sc_d[:, :])
            a_col = sc[:, 0:1]
            k1_col = sc[:, 1:2]

            if MM_DTYPE == "f32r":
                ma_mm = ma[:, :].bitcast(mybir.dt.float32r)
                mb_mm = mb[:, :].bitcast(mybir.dt.float32r)
            else:
                ma_mm = ma[:, :]
                mb_mm = mb[:, :]

            for i in range(ITERS):
                sl = slice(i * NT, (i + 1) * NT)
                x = pool.tile([P, NT], F32)
                nc.sync.dma_start(
                    x[:, :], xt_v[:, :, sl]
                )
                y = pool.tile([P, NT], F32)
                nc.sync.dma_start(
                    y[:, :], x0_v[:, :, sl]
                )

                x_mm = x[:, :].bitcast(mybir.dt.float32r) if MM_DTYPE == "f32r" else x

                # dn = kron(I4, a*M)^T @ x   (per group: a * M^T x)
                dn = psum.tile([P, NT], F32, tag="dn")
                for j in range(NT // MM_N):
                    js = slice(j * MM_N, (j + 1) * MM_N)
                    nc.tensor.matmul(dn[:, js], ma_mm, x_mm[:, js], start=True, stop=True)

                # rden = 1 / (dn + k1)
                rden = pool.tile([P, NT], F32, bufs=2)
                if DIV_MODE == "lnexp":
                    lnd = pool.tile([P, NT], F32, bufs=2)
                    nc.scalar.activation(lnd[:, :], dn[:, :], AF.Ln, bias=k1_col, scale=1.0)
                    nc.scalar.activation(rden[:, :], lnd[:, :], AF.Exp, bias=0.0, scale=-1.0)
                else:
                    den = pool.tile([P, NT], F32, bufs=2)
                    nc.scalar.activation(den[:, :], dn[:, :], AF.Identity, bias=k1_col, scale=1.0)
                    nc.vector.reciprocal_approx_fast(out=rden[:, :], in_=den[:, :])

                # g = x0 * rden
                g = pool.tile([P, NT], F32, bufs=2)
                nc.vector.tensor_tensor(g[:, :], y[:, :], rden[:, :], mybir.AluOpType.mult)

                g_mm = g[:, :].bitcast(mybir.dt.float32r) if MM_DTYPE == "f32r" else g

                # r = kron(I4, M)^T @ g      (per group: M g, M symmetric)
                r = psum.tile([P, NT], F32, tag="r")
                for j in range(NT // MM_N):
                    js = slice(j * MM_N, (j + 1) * MM_N)
                    nc.tensor.matmul(r[:, js], mb_mm, g_mm[:, js], start=True, stop=True)

                # out = (a*x + k1) * r
                o = pool.tile([P, NT], F32, bufs=2)
                acc = pool.tile([P, 1], F32, bufs=2, tag="acc")
                nc.vector.affine_mul_reduce(
                    out=o[:, :], accum_out=acc[:, :], in0=x[:, :], in1=r[:, :],
                    scale=a_col, bias=k1_col,
                )

                nc.sync.dma_start(
                    out_v[:, :, sl], o[:, :]
                )

    nc.compile()
    _CACHE[key] = nc
    return nc


def _host_prep(inputs):
    xt = np.ascontiguousarray(np.asarray(inputs["xt"], dtype=np.float32))
    x0 = np.ascontiguousarray(np.asarray(inputs["theta_x0"], dtype=np.float32))
    t = np.asarray(inputs["t"]).astype(np.int64)
    al = np.asarray(inputs["alphas"], dtype=np.float32)
    cu = np.asarray(inputs["cumalphas"], dtype=np.float32)

    eyeC = np.eye(C, dtype=np.float64)
    eyeG = np.eye(G, dtype=np.float64)
    in_maps = []
    for b in range(B):
        tm = int(t[b]) - 1
        a = 0.0 if tm == 0 else float(al[tm])
        ca = 1.0 if tm == 0 else float(cu[tm - 1])
        u = (1.0 - ca) / C
        k1 = (1.0 - a) / C
        M = ca * eyeC + u
        ma = np.kron(eyeG, a * M).astype(np.float32)
        mb = np.kron(eyeG, M).astype(np.float32)
        sc = np.empty((P, 2), dtype=np.float32)
        sc[:, 0] = a
        sc[:, 1] = k1
        in_maps.append(
            {
                "xt": xt[b].reshape(C, HW),
                "x0": x0[b].reshape(C, HW),
                "ma": ma,
                "mb": mb,
                "sc": sc,
            }
        )
    return in_maps


def _run(inputs, trace=False, **kw):
    nc = _build()
    in_maps = _host_prep(inputs)
    res = run_bass_kernel_spmd(
        nc, in_maps, core_ids=list(range(NCORES)), trace=trace, **kw
    )
    out = np.stack([r["out"].reshape(C, H, W) for r in res.results])
    return out, res


def kernel(**inputs):
    out, _ = _run(inputs, trace=False)
    return out
